# revision 37
# baseline (speedup 1.0000x reference)
"""Trainium2 Bass kernel for the fused attention+LN+GELU+projection module.

Shapes (hardcoded): x [B=256, S=512, D=512]; k/q/v_w [H=256, D]; attn_bias [S, H];
out_w [D, S*H]; output [B, 1, D].

Distribution across 8 NeuronCores:
 - phases 1-7 (QKV proj, scores, softmax, apply, +bias, LN, GELU): data-parallel
   over batch, 32 batches/core, bf16 matmul path (fp32 PSUM accumulate).
 - phase 8 (y = act @ out_w.T): contraction dim S*H sharded 8 ways; the batch
   dim is split into 2 chunks of 16 local batches each with one AllToAll per
   chunk so the first exchange overlaps the second half of attention. Received
   [batch, sh] blocks are transposed to [sh, batch] via XBAR DMA-transpose and
   multiplied against this core's 1/8 slice of out_w. Per-core partial outputs
   are summed on the host (no device AllReduce).
"""

import sys

sys.path.insert(0, "/opt/trn_rl_repo")

import numpy as np
import ml_dtypes

import concourse.bacc as bacc
import concourse.tile as tile
from concourse import mybir
from concourse.bass_utils import run_bass_kernel_spmd
from concourse.hw_specs import get_activation_tables
from concourse.tile_rust import add_dep_helper
from concourse.dve_ops import (
    RECIP_APPROX_FAST_CONSTS,
    RECIPROCAL_APPROX_FAST,
)
import bass_rust as _bass_rust

N_CORES = 8
B, S, H, D = 256, 512, 256, 512
NB = B // N_CORES          # batches per core
SCALE = 1.0 / (B ** 0.5)   # score scale (batch-size based, faithful to ref)
LN_EPS = 1e-5
NDT = D // 128             # 4 d-tiles
NST = S // 128             # 4 s-tiles
NHT = H // 128             # 2 h-tiles
SREM = S // N_CORES        # 64 s rows per core contraction slice
SLICE = SREM * H           # 16384 contraction elems per core
NC_T = SLICE // 128        # 128 contraction tiles per core
G = 8                      # ACT-table batch group size
CH = 2                     # AllToAll chunks
GB = NB // CH              # local batches per chunk (16)
CW = 2048                  # sh columns per DMA-transpose slab
NSLAB = SLICE // CW        # 8 slabs per chunk
TPS = CW // 128            # 16 contraction tiles per slab
OW_BUFS = 62               # out_w tiles resident across both phase-8 passes

F32 = mybir.dt.float32
F32R = mybir.dt.float32r
BF16 = mybir.dt.bfloat16
AF = mybir.ActivationFunctionType
BF = ml_dtypes.bfloat16


class _Bacc(bacc.Bacc):
    """Bacc whose activation-table binding is restricted so that exp/ln are
    only servable by natural_log_exp_and_others and gelu by gelu_and_others.
    Avoids per-op ACT_TABLE_LOAD thrash (~2.7us each) from the default
    first-match binding. Table ids keep their act_info.json order."""

    def insert_act_table_loads(self):
        has_activation = any(
            isinstance(i, mybir.InstActivation)
            for b in self.main_func.blocks
            for i in b.instructions
        )
        if not has_activation:
            return
        keep = {"natural_log_exp_and_others", "gelu_and_others"}
        strip = {AF.Exp, AF.Ln, AF.Gelu}
        tables = []
        for name, funcs in get_activation_tables(self.m.arch).items():
            if name not in keep:
                funcs = funcs - strip
            tables.append((name, funcs))
        _bass_rust.insert_act_table_loads(self, tables)


def _build(ln_trivial: bool):
    nc = _Bacc("TRN2", target_bir_lowering=False, debug=False,
               num_devices=N_CORES)

    # ---- DRAM I/O ----
    xT = nc.dram_tensor("xT", [NB, NDT, 128, S], BF16, kind="ExternalInput").ap()
    kq_wT = nc.dram_tensor("kq_wT", [NDT, 128, 2 * H], BF16, kind="ExternalInput").ap()
    v_wT = nc.dram_tensor("v_wT", [NDT, 128, H], BF16, kind="ExternalInput").ap()
    kq_b = nc.dram_tensor("kq_b", [128, 2 * H], F32, kind="ExternalInput").ap()
    v_b2 = nc.dram_tensor("v_b2", [NHT, 128, 1], F32, kind="ExternalInput").ap()
    ab = nc.dram_tensor("ab", [NST, 128, H], F32, kind="ExternalInput").ap()
    ones_bf = nc.dram_tensor("ones_bf", [128, 128], BF16, kind="ExternalInput").ap()
    ones_f32 = nc.dram_tensor("ones_f32", [128, 128], F32R, kind="ExternalInput").ap()
    owT = nc.dram_tensor("owT", [NC_T, 128, D], BF16, kind="ExternalInput").ap()
    if not ln_trivial:
        lng = nc.dram_tensor("lng", [128, H], F32, kind="ExternalInput").ap()
        lnb = nc.dram_tensor("lnb", [128, H], F32, kind="ExternalInput").ap()
    y_out = nc.dram_tensor("y", [CH, 128, D], F32, kind="ExternalOutput").ap()

    # internal DRAM (collective bounce buffers), one pair per chunk
    a2a_in = [nc.dram_tensor(f"a2a_in{k}", [N_CORES, GB, SREM, H], BF16).ap()
              for k in range(CH)]
    a2a_out = [nc.dram_tensor(f"a2a_out{k}", [N_CORES * GB, SLICE], BF16).ap()
               for k in range(CH)]

    from contextlib import ExitStack
    with tile.TileContext(nc) as tc:
        with ExitStack() as stack:
            pool = lambda *a, **kw: stack.enter_context(tc.tile_pool(*a, **kw))
            constp = pool(name="const", bufs=1)
            xtp = pool(name="xt", bufs=14)
            kqp = pool(name="kqsb", bufs=19)
            vtp = pool(name="vtsb", bufs=10)
            ep = pool(name="esb", bufs=6)
            wp = pool(name="wsb", bufs=6)
            tp = pool(name="tsb", bufs=33)
            actp = pool(name="actsb", bufs=36)
            statp = pool(name="stat", bufs=16)
            lnstatp = pool(name="lnstat", bufs=40)
            recp = pool(name="rec", bufs=3)
            owp = pool(name="p8ow", bufs=OW_BUFS)
            atp = pool(name="p8at", bufs=9)
            ysbp = pool(name="ysb", bufs=2)
            # PSUM slots are bank-granular (2KB/partition each, 8 banks).
            # bigps holds the [128,512] projection accumulators (and is
            # reused for phase-8's ypsum); pairps packs two logical
            # [128,256] tiles per bank: scores ht0|ht1, sm|bc, p5 st-pairs.
            bigps = pool(name="bigps", bufs=4, space="PSUM")
            scps = pool(name="scps", bufs=2, space="PSUM")
            pairps = pool(name="pairps", bufs=2, space="PSUM")
            # ---- persistent constants ----
            kqw_sb = []
            vw_sb = []
            for dt_ in range(NDT):
                t = constp.tile([128, 2 * H], BF16, tag=f"kqw{dt_}")
                nc.sync.dma_start(t[:], kq_wT[dt_])
                kqw_sb.append(t)
                t = constp.tile([128, H], BF16, tag=f"vw{dt_}")
                nc.sync.dma_start(t[:], v_wT[dt_])
                vw_sb.append(t)
            kqb_sb = constp.tile([128, 2 * H], F32, tag="kqb")
            nc.sync.dma_start(kqb_sb[:], kq_b[:])
            vb_sb = []
            for ht in range(NHT):
                t = constp.tile([128, 1], F32, tag=f"vb{ht}")
                nc.sync.dma_start(t[:], v_b2[ht])
                vb_sb.append(t)
            ab_sb = []
            for st in range(NST):
                t = constp.tile([128, H], F32, tag=f"ab{st}")
                nc.sync.dma_start(t[:], ab[st])
                ab_sb.append(t)
            if not ln_trivial:
                lng_sb = constp.tile([128, H], F32, tag="lng")
                nc.sync.dma_start(lng_sb[:], lng[:])
                lnb_sb = constp.tile([128, H], F32, tag="lnb")
                nc.sync.dma_start(lnb_sb[:], lnb[:])
            ones_sb = constp.tile([128, 128], BF16, tag="ones")
            nc.sync.dma_start(ones_sb[:], ones_bf[:])
            ones_col = ones_sb[:, 0:1]
            ones_r32 = constp.tile([128, 128], F32R, tag="ones_r32")
            nc.sync.dma_start(ones_r32[:], ones_f32[:])
            ones_row_r = ones_r32[0:1, :]
            eps_sb = constp.tile([128, 1], F32, tag="eps")
            nc.gpsimd.memset(eps_sb[:], LN_EPS)

            at_slabs = [[None] * NSLAB for _ in range(CH)]

            # ---- software-pipelined attention over batches ----
            # Per-batch work is split into stages staggered across loop
            # iterations so that no engine queue ever sits at its head
            # waiting on a freshly-issued cross-engine dependency:
            #   iteration b emits  denom(b-1) | proj(b) | bc(b-1) |
            #   scores(b) | apply+stats(b-1) | exp(b) | ln/rstd(b-1)
            # (at group boundaries exp(b) moves after the GELU pass so the
            # scalar queue order matches the ACT-table dependency chain).
            st_ = {}             # per-batch live tiles
            pend = []            # deferred-GELU state per batch in group
            grp_tbl_insts = []   # this group's exp/ln ACT instructions
            prev_gelu = None     # last gelu instruction of previous group

            def emit_proj(b):
                xt = []
                for dt_ in range(NDT):
                    t = xtp.tile([128, S], BF16, tag="xt")
                    nc.sync.dma_start(t[:], xT[b, dt_])
                    xt.append(t)
                # vT[h, s] = sum_d v_wT[d, h] * xT[d, s]  (+v_b per-part,
                # applied by the Scalar ACT during the PSUM->SBUF copy)
                vt_sb = []
                for ht in range(NHT):
                    ps = bigps.tile([128, S], F32, tag="bigps")
                    for dt_ in range(NDT):
                        nc.tensor.matmul(
                            ps[:], vw_sb[dt_][:, ht * 128:(ht + 1) * 128],
                            xt[dt_][:],
                            start=(dt_ == 0), stop=(dt_ == NDT - 1))
                    t = vtp.tile([128, S], BF16, tag="vt")
                    nc.scalar.activation(t[:], ps[:], AF.Identity,
                                         bias=vb_sb[ht][:])
                    vt_sb.append(t)
                # kq[s, j] = sum_d x[s, d] * [k_wT | q_wT][d, j]  (+bias)
                kq_sb = []
                for stt in range(NST):
                    ps = bigps.tile([128, 2 * H], F32, tag="bigps")
                    for dt_ in range(NDT):
                        nc.tensor.matmul(
                            ps[:], xt[dt_][:, stt * 128:(stt + 1) * 128],
                            kqw_sb[dt_][:],
                            start=(dt_ == 0), stop=(dt_ == NDT - 1))
                    t = kqp.tile([128, 2 * H], BF16, tag="kq")
                    nc.vector.tensor_add(t[:], ps[:], kqb_sb[:])
                    kq_sb.append(t)
                st_[b] = {"vt": vt_sb, "kq": kq_sb}

            def emit_scores(b):
                s = st_[b]
                kq_sb = s["kq"]
                s["sc"] = []
                for ht in range(NHT):
                    sc = scps.tile([128, H], F32, tag="scps")
                    for stt in range(NST):
                        nc.tensor.matmul(
                            sc[:],
                            kq_sb[stt][:, ht * 128:(ht + 1) * 128],
                            kq_sb[stt][:, H:2 * H],
                            start=(stt == 0), stop=(stt == NST - 1))
                    s["sc"].append(sc)

            def emit_exp(b):
                s = st_[b]
                e_sb = []
                for ht in range(NHT):
                    t = ep.tile([128, H], BF16, tag="e")
                    ei = nc.scalar.activation(
                        t[:], s["sc"][ht][:], AF.Exp,
                        scale=SCALE)
                    grp_tbl_insts.append(ei)
                    e_sb.append(t)
                s["e"] = e_sb

            def emit_denom(b):
                # softmax denom over h (partition dim) via ones-matmuls;
                # approx reciprocal written straight into an f32r tile so
                # the broadcast matmul needs no dtype-convert copy.
                s = st_[b]
                smbc = pairps.tile([128, 2 * H], F32, tag="pair")
                for ht in range(NHT):
                    nc.tensor.matmul(smbc[0:1, 0:H], ones_col, s["e"][ht][:],
                                     start=(ht == 0), stop=(ht == NHT - 1))
                rec_sb = recp.tile([1, H], F32R, tag="rec")
                c = RECIP_APPROX_FAST_CONSTS
                nc.vector._custom_dve(
                    RECIPROCAL_APPROX_FAST, out=rec_sb[:],
                    in0=smbc[0:1, 0:H], s0=c["s0"], s1=c["s1"],
                    imm2=c["imm2"])
                s["smbc"] = smbc
                s["rec"] = rec_sb

            def emit_bc(b):
                s = st_[b]
                bcp = s["smbc"][:, H:2 * H]
                nc.tensor.matmul(bcp, ones_row_r, s["rec"][:],
                                 start=True, stop=True)
                w_sb = []
                for ht in range(NHT):
                    t = wp.tile([128, H], BF16, tag="w")
                    nc.vector.tensor_mul(t[:], s["e"][ht][:], bcp)
                    w_sb.append(t)
                s["w"] = w_sb

            def emit_apply(b):
                # out5[s, g] = sum_h vT[h, s] w[h, g]; +attn_bias; LN stats.
                # p5 tiles pack two st halves per PSUM bank; per-batch
                # mean/var collect into one [128, NST, 2] tile so the
                # ln/rstd pass is 2 ACTs (not 8).
                s = st_[b]
                tl = []
                mva = statp.tile([128, NST, 2], F32, tag="mva")
                for pp in range(NST // 2):
                    p5 = pairps.tile([128, 2 * H], F32, tag="pair")
                    for half in range(2):
                        stt = 2 * pp + half
                        for ht in range(NHT):
                            nc.tensor.matmul(
                                p5[:, half * H:(half + 1) * H],
                                s["vt"][ht][:, stt * 128:(stt + 1) * 128],
                                s["w"][ht][:],
                                start=(ht == 0), stop=(ht == NHT - 1))
                    for half in range(2):
                        stt = 2 * pp + half
                        t_sb = tp.tile([128, H], BF16, tag="t")
                        nc.vector.tensor_add(
                            t_sb[:], p5[:, half * H:(half + 1) * H],
                            ab_sb[stt][:])
                        st6 = statp.tile([128, 6], F32, tag="st6")
                        nc.vector.bn_stats(st6[:], t_sb[:])
                        nc.vector.bn_aggr(mva[:, stt], st6[:])
                        tl.append(t_sb)
                s["t"] = tl
                s["mva"] = mva

            def emit_lnrstd(b):
                # rstd = (var+eps)^-0.5 = exp(-0.5*ln(var+eps)) on all NST
                # tiles at once (strided var columns); nb = -mu*rstd on the
                # otherwise-idle GpSimd engine.
                s = st_[b]
                mva = s["mva"]
                lnv = lnstatp.tile([128, NST], F32, tag="lnv")
                li = nc.scalar.activation(lnv[:], mva[:, :, 1:2], AF.Ln,
                                          bias=eps_sb[:])
                grp_tbl_insts.append(li)
                rstd = lnstatp.tile([128, NST], F32, tag="rstd")
                ri = nc.scalar.activation(rstd[:], lnv[:], AF.Exp,
                                          scale=-0.5)
                grp_tbl_insts.append(ri)
                nb_t = lnstatp.tile([128, NST], F32, tag="nb")
                nc.vector.scalar_tensor_tensor(
                    nb_t[:], mva[:, :, 0:1], -1.0, rstd[:],
                    mybir.AluOpType.mult, mybir.AluOpType.mult)
                pend.append((b, s["t"], rstd, nb_t))
                del st_[b]

            def emit_gelu_group():
                nonlocal prev_gelu, grp_tbl_insts, pend
                if prev_gelu is not None:
                    # keep ACT table phases disjoint across groups
                    for inst in grp_tbl_insts:
                        add_dep_helper(inst.ins, prev_gelu.ins,
                                       sync=False,
                                       reason="act-table grouping")
                last_tbl = grp_tbl_insts[-1]
                grp_tbl_insts = []
                for pb, tl, rstd, nb_t in pend:
                    ck = pb // GB
                    lb = pb % GB
                    for stt in range(NST):
                        act_sb = actp.tile([128, H], BF16, tag="act")
                        if ln_trivial:
                            gi = nc.scalar.activation(
                                act_sb[:], tl[stt][:], AF.Gelu,
                                bias=nb_t[:, stt:stt + 1],
                                scale=rstd[:, stt:stt + 1])
                        else:
                            nrm = tp.tile([128, H], F32, tag="nrm")
                            nc.scalar.activation(
                                nrm[:], tl[stt][:], AF.Identity,
                                bias=nb_t[:, stt:stt + 1],
                                scale=rstd[:, stt:stt + 1])
                            nc.vector.tensor_mul(nrm[:], nrm[:], lng_sb[:])
                            nc.vector.tensor_add(nrm[:], nrm[:], lnb_sb[:])
                            gi = nc.scalar.activation(
                                act_sb[:], nrm[:], AF.Gelu)
                        add_dep_helper(gi.ins, last_tbl.ins,
                                       sync=False,
                                       reason="act-table grouping")
                        # single DMA covering both destination shards;
                        # split across Scalar and GpSimd queues
                        dst = a2a_in[ck][2 * stt:2 * stt + 2, lb]
                        if ck == 0:
                            nc.gpsimd.dma_start(dst, act_sb[:])
                        else:
                            # the AllToAll trigger blocks the GpSimd queue
                            # until the collective completes, so chunk-1
                            # writes must not queue behind chunk-0's trigger
                            nc.sync.dma_start(dst, act_sb[:])
                        prev_gelu = gi
                pend = []

            proj_done = set()
            ow_tiles = [None] * NC_T

            def prefetch_ow(c):
                t = owp.tile([128, D], BF16, tag="ow")
                nc.gpsimd.dma_start(t[:], owT[c])
                ow_tiles[c] = t

            def proj(b):
                if b < NB and b not in proj_done:
                    emit_proj(b)
                    proj_done.add(b)

            for b in range(NB + 1):
                boundary = b > 0 and b % G == 0
                post_boundary = b > 1 and b % G == 1
                if post_boundary:
                    # right after a GELU pass the Scalar queue is still
                    # draining the pass + next exp; give the PE two batches
                    # of exp-independent projection work first
                    proj(b)
                    proj(b + 1)
                    emit_scores(b)
                    emit_exp(b)
                    emit_denom(b - 1)
                    emit_bc(b - 1)
                else:
                    if b > 0:
                        emit_denom(b - 1)
                    proj(b)
                    if b > 0:
                        emit_bc(b - 1)
                    if b < NB:
                        emit_scores(b)
                        if not boundary:
                            emit_exp(b)
                if b > 0:
                    emit_apply(b - 1)
                    emit_lnrstd(b - 1)
                    if boundary:
                        emit_gelu_group()
                        if b < NB:
                            emit_exp(b)
                    if b % GB == 0:
                        ck = b // GB - 1
                        nc.gpsimd.collective_compute(
                            "AllToAll", mybir.AluOpType.bypass,
                            replica_groups=[list(range(N_CORES))],
                            ins=[a2a_in[ck].opt()],
                            outs=[a2a_out[ck].opt()])


            # ---- phase 8: y_part[b, d] = sum_sh actT[sh, b] * owT[sh, d] ----
            # XBAR DMA-transpose slabs [128 batch, CW sh] -> [128 sh, TPS,
            # 128 batch] run on the Scalar queue (idle after attention) so
            # their collective-wait never blocks Sync's ow streaming.
            def emit_slabs(ck, reverse=False):
                order = range(NSLAB - 1, -1, -1) if reverse else range(NSLAB)
                for c8 in order:
                    at = atp.tile([128, TPS, 128], BF16, tag="at")
                    nc.scalar.dma_start_transpose(
                        at[:], a2a_out[ck][0:128, c8 * CW:(c8 + 1) * CW])
                    at_slabs[ck][c8] = at

            emit_slabs(0)
            ypsum = []
            n_fresh = NC_T - OW_BUFS
            # chunk 0: stream all of out_w through owp; the last OW_BUFS
            # tiles stay resident for chunk 1.
            yp_t = bigps.tile([128, D], F32, tag="bigps")
            ypsum.append(yp_t)
            for c in range(NC_T):
                ow_t = owp.tile([128, D], BF16, tag="ow")
                # stream out_w on both free queues so chunk 0 is PE-bound,
                # not DMA-bound (Scalar is idle after the chunk-0 slabs)
                if c % 2 == 0:
                    nc.sync.dma_start(ow_t[:], owT[c])
                else:
                    nc.scalar.dma_start(ow_t[:], owT[c])
                ow_tiles[c] = ow_t
                nc.tensor.matmul(
                    yp_t[:], at_slabs[0][c // TPS][:, c % TPS, :], ow_t[:],
                    start=(c == 0), stop=(c == NC_T - 1))
            # chunk 1: the last OW_BUFS tiles are still resident (process
            # them newest-first so their slots are fully read before the
            # re-streamed tiles rotate in); the re-streamed loads are
            # emitted first so they prefetch during the second AllToAll.
            emit_slabs(1)
            yp_t = bigps.tile([128, D], F32, tag="bigps")
            ypsum.append(yp_t)
            order = (list(range(NC_T - 1, n_fresh - 1, -1))
                     + list(range(n_fresh - 1, -1, -1)))
            for i, c in enumerate(order):
                if c >= n_fresh:
                    ow_t = ow_tiles[c]
                else:
                    ow_t = owp.tile([128, D], BF16, tag="ow")
                    nc.sync.dma_start(ow_t[:], owT[c])
                nc.tensor.matmul(
                    yp_t[:], at_slabs[1][c // TPS][:, c % TPS, :], ow_t[:],
                    start=(i == 0), stop=(i == NC_T - 1))
            for ck in range(CH):
                y_sb = ysbp.tile([128, D], F32, tag="ysb")
                nc.vector.tensor_copy(y_sb[:], ypsum[ck][:])
                nc.sync.dma_start(y_out[ck], y_sb[:])

    nc.compile()
    return nc


_CACHE = {}


def _get_program(ln_trivial):
    if ln_trivial not in _CACHE:
        _CACHE[ln_trivial] = _build(ln_trivial)
    return _CACHE[ln_trivial]


def _prep_inputs(x, k_w, k_b, q_w, q_b, v_w, v_b, attn_bias, ln_g, ln_b,
                 out_w, out_b):
    ln_trivial = bool(np.all(ln_g == 1.0) and np.all(ln_b == 0.0))
    kq_wT = np.ascontiguousarray(
        np.concatenate([k_w.T, q_w.T], axis=1)).reshape(
            NDT, 128, 2 * H).astype(BF)
    v_wT = np.ascontiguousarray(v_w.T).reshape(NDT, 128, H).astype(BF)
    kq_b = np.ascontiguousarray(
        np.tile(np.concatenate([k_b, q_b])[None, :], (128, 1)))
    v_b2 = np.ascontiguousarray(v_b.reshape(NHT, 128, 1))
    ab = np.ascontiguousarray(attn_bias.reshape(NST, 128, H))
    owT_full = np.ascontiguousarray(out_w.T)  # [S*H, D]
    shared = dict(kq_wT=kq_wT, v_wT=v_wT, kq_b=kq_b, v_b2=v_b2, ab=ab,
                  ones_bf=np.ones((128, 128), BF),
                  ones_f32=np.ones((128, 128), np.float32))
    if not ln_trivial:
        shared["lng"] = np.ascontiguousarray(np.tile(ln_g[None, :], (128, 1)))
        shared["lnb"] = np.ascontiguousarray(np.tile(ln_b[None, :], (128, 1)))
    in_maps = []
    for i in range(N_CORES):
        xi = np.ascontiguousarray(
            x[i * NB:(i + 1) * NB].transpose(0, 2, 1)).reshape(
                NB, NDT, 128, S).astype(BF)
        owi = np.ascontiguousarray(
            owT_full[i * SLICE:(i + 1) * SLICE]).reshape(
                NC_T, 128, D).astype(BF)
        m = dict(shared)
        m["xT"] = xi
        m["owT"] = owi
        in_maps.append(m)
    return ln_trivial, in_maps


def _postprocess(res, out_b):
    # each core returns its partial projection [CH, 128, D]; sum over cores
    ysum = np.zeros((CH, N_CORES * GB, D), np.float32)
    for r in res.results:
        ysum += np.asarray(r["y"], np.float32)
    # a2a_out row r of chunk k is global batch (r//GB)*NB + k*GB + (r%GB)
    rr = np.arange(N_CORES * GB)
    y = np.empty((B, D), np.float32)
    for k in range(CH):
        y[(rr // GB) * NB + k * GB + (rr % GB)] = ysum[k]
    y += out_b[None, :]
    return y.reshape(B, 1, D).astype(np.float32)


def kernel(**inputs):
    xs = {k: np.asarray(v, dtype=np.float32) for k, v in inputs.items()}
    ln_trivial, in_maps = _prep_inputs(
        xs["x"], xs["k_w"], xs["k_b"], xs["q_w"], xs["q_b"], xs["v_w"],
        xs["v_b"], xs["attn_bias"], xs["ln_g"], xs["ln_b"], xs["out_w"],
        xs["out_b"])
    nc = _get_program(ln_trivial)
    res = run_bass_kernel_spmd(nc, in_maps, core_ids=list(range(N_CORES)))
    return _postprocess(res, xs["out_b"])


# revision 38
# speedup vs baseline: 1.0044x; 1.0044x over previous
"""Trainium2 Bass kernel for the fused attention+LN+GELU+projection module.

Shapes (hardcoded): x [B=256, S=512, D=512]; k/q/v_w [H=256, D]; attn_bias [S, H];
out_w [D, S*H]; output [B, 1, D].

Distribution across 8 NeuronCores:
 - phases 1-7 (QKV proj, scores, softmax, apply, +bias, LN, GELU): data-parallel
   over batch, 32 batches/core, bf16 matmul path (fp32 PSUM accumulate).
 - phase 8 (y = act @ out_w.T): contraction dim S*H sharded 8 ways; the batch
   dim is split into 2 chunks of 16 local batches each with one AllToAll per
   chunk so the first exchange overlaps the second half of attention. Received
   [batch, sh] blocks are transposed to [sh, batch] via XBAR DMA-transpose and
   multiplied against this core's 1/8 slice of out_w. Per-core partial outputs
   are summed on the host (no device AllReduce).
"""

import sys

sys.path.insert(0, "/opt/trn_rl_repo")

import numpy as np
import ml_dtypes

import concourse.bacc as bacc
import concourse.tile as tile
from concourse import mybir
from concourse.bass_utils import run_bass_kernel_spmd
from concourse.hw_specs import get_activation_tables
from concourse.tile_rust import add_dep_helper
from concourse.dve_ops import (
    RECIP_APPROX_FAST_CONSTS,
    RECIPROCAL_APPROX_FAST,
)
import bass_rust as _bass_rust

N_CORES = 8
B, S, H, D = 256, 512, 256, 512
NB = B // N_CORES          # batches per core
SCALE = 1.0 / (B ** 0.5)   # score scale (batch-size based, faithful to ref)
LN_EPS = 1e-5
NDT = D // 128             # 4 d-tiles
NST = S // 128             # 4 s-tiles
NHT = H // 128             # 2 h-tiles
SREM = S // N_CORES        # 64 s rows per core contraction slice
SLICE = SREM * H           # 16384 contraction elems per core
NC_T = SLICE // 128        # 128 contraction tiles per core
G = 8                      # ACT-table batch group size
CH = 2                     # AllToAll chunks
GB = NB // CH              # local batches per chunk (16)
CW = 2048                  # sh columns per DMA-transpose slab
NSLAB = SLICE // CW        # 8 slabs per chunk
TPS = CW // 128            # 16 contraction tiles per slab
OW_BUFS = 68               # out_w tiles resident across both phase-8 passes

F32 = mybir.dt.float32
F32R = mybir.dt.float32r
BF16 = mybir.dt.bfloat16
AF = mybir.ActivationFunctionType
BF = ml_dtypes.bfloat16


class _Bacc(bacc.Bacc):
    """Bacc whose activation-table binding is restricted so that exp/ln are
    only servable by natural_log_exp_and_others and gelu by gelu_and_others.
    Avoids per-op ACT_TABLE_LOAD thrash (~2.7us each) from the default
    first-match binding. Table ids keep their act_info.json order."""

    def insert_act_table_loads(self):
        has_activation = any(
            isinstance(i, mybir.InstActivation)
            for b in self.main_func.blocks
            for i in b.instructions
        )
        if not has_activation:
            return
        keep = {"natural_log_exp_and_others", "gelu_and_others"}
        strip = {AF.Exp, AF.Ln, AF.Gelu}
        tables = []
        for name, funcs in get_activation_tables(self.m.arch).items():
            if name not in keep:
                funcs = funcs - strip
            tables.append((name, funcs))
        _bass_rust.insert_act_table_loads(self, tables)


def _build(ln_trivial: bool):
    nc = _Bacc("TRN2", target_bir_lowering=False, debug=False,
               num_devices=N_CORES)

    # ---- DRAM I/O ----
    xT = nc.dram_tensor("xT", [NB, NDT, 128, S], BF16, kind="ExternalInput").ap()
    kq_wT = nc.dram_tensor("kq_wT", [NDT, 128, 2 * H], BF16, kind="ExternalInput").ap()
    v_wT = nc.dram_tensor("v_wT", [NDT, 128, H], BF16, kind="ExternalInput").ap()
    kq_b = nc.dram_tensor("kq_b", [128, 2 * H], F32, kind="ExternalInput").ap()
    v_b2 = nc.dram_tensor("v_b2", [NHT, 128, 1], F32, kind="ExternalInput").ap()
    ab = nc.dram_tensor("ab", [NST, 128, H], F32, kind="ExternalInput").ap()
    ones_bf = nc.dram_tensor("ones_bf", [128, 128], BF16, kind="ExternalInput").ap()
    ones_f32 = nc.dram_tensor("ones_f32", [128, 128], F32R, kind="ExternalInput").ap()
    owT = nc.dram_tensor("owT", [NC_T, 128, D], BF16, kind="ExternalInput").ap()
    if not ln_trivial:
        lng = nc.dram_tensor("lng", [128, H], F32, kind="ExternalInput").ap()
        lnb = nc.dram_tensor("lnb", [128, H], F32, kind="ExternalInput").ap()
    y_out = nc.dram_tensor("y", [CH, 128, D], F32, kind="ExternalOutput").ap()

    # internal DRAM (collective bounce buffers), one pair per chunk
    a2a_in = [nc.dram_tensor(f"a2a_in{k}", [N_CORES, GB, SREM, H], BF16).ap()
              for k in range(CH)]
    a2a_out = [nc.dram_tensor(f"a2a_out{k}", [N_CORES * GB, SLICE], BF16).ap()
               for k in range(CH)]

    from contextlib import ExitStack
    with tile.TileContext(nc) as tc:
        with ExitStack() as stack:
            pool = lambda *a, **kw: stack.enter_context(tc.tile_pool(*a, **kw))
            constp = pool(name="const", bufs=1)
            xtp = pool(name="xt", bufs=14)
            kqp = pool(name="kqsb", bufs=16)
            vtp = pool(name="vtsb", bufs=8)
            ep = pool(name="esb", bufs=6)
            wp = pool(name="wsb", bufs=6)
            tp = pool(name="tsb", bufs=33)
            actp = pool(name="actsb", bufs=36)
            statp = pool(name="stat", bufs=16)
            lnstatp = pool(name="lnstat", bufs=40)
            recp = pool(name="rec", bufs=3)
            owp = pool(name="p8ow", bufs=OW_BUFS)
            atp = pool(name="p8at", bufs=9)
            ysbp = pool(name="ysb", bufs=2)
            # PSUM slots are bank-granular (2KB/partition each, 8 banks).
            # bigps holds the [128,512] projection accumulators (and is
            # reused for phase-8's ypsum); pairps packs two logical
            # [128,256] tiles per bank: scores ht0|ht1, sm|bc, p5 st-pairs.
            bigps = pool(name="bigps", bufs=4, space="PSUM")
            scps = pool(name="scps", bufs=2, space="PSUM")
            pairps = pool(name="pairps", bufs=2, space="PSUM")
            # ---- persistent constants ----
            kqw_sb = []
            vw_sb = []
            for dt_ in range(NDT):
                t = constp.tile([128, 2 * H], BF16, tag=f"kqw{dt_}")
                nc.sync.dma_start(t[:], kq_wT[dt_])
                kqw_sb.append(t)
                t = constp.tile([128, H], BF16, tag=f"vw{dt_}")
                nc.sync.dma_start(t[:], v_wT[dt_])
                vw_sb.append(t)
            kqb_sb = constp.tile([128, 2 * H], F32, tag="kqb")
            nc.sync.dma_start(kqb_sb[:], kq_b[:])
            vb_sb = []
            for ht in range(NHT):
                t = constp.tile([128, 1], F32, tag=f"vb{ht}")
                nc.sync.dma_start(t[:], v_b2[ht])
                vb_sb.append(t)
            ab_sb = []
            for st in range(NST):
                t = constp.tile([128, H], F32, tag=f"ab{st}")
                nc.sync.dma_start(t[:], ab[st])
                ab_sb.append(t)
            if not ln_trivial:
                lng_sb = constp.tile([128, H], F32, tag="lng")
                nc.sync.dma_start(lng_sb[:], lng[:])
                lnb_sb = constp.tile([128, H], F32, tag="lnb")
                nc.sync.dma_start(lnb_sb[:], lnb[:])
            ones_sb = constp.tile([128, 128], BF16, tag="ones")
            nc.sync.dma_start(ones_sb[:], ones_bf[:])
            ones_col = ones_sb[:, 0:1]
            ones_r32 = constp.tile([128, 128], F32R, tag="ones_r32")
            nc.sync.dma_start(ones_r32[:], ones_f32[:])
            ones_row_r = ones_r32[0:1, :]
            eps_sb = constp.tile([128, 1], F32, tag="eps")
            nc.gpsimd.memset(eps_sb[:], LN_EPS)

            at_slabs = [[None] * NSLAB for _ in range(CH)]

            # ---- software-pipelined attention over batches ----
            # Per-batch work is split into stages staggered across loop
            # iterations so that no engine queue ever sits at its head
            # waiting on a freshly-issued cross-engine dependency:
            #   iteration b emits  denom(b-1) | proj(b) | bc(b-1) |
            #   scores(b) | apply+stats(b-1) | exp(b) | ln/rstd(b-1)
            # (at group boundaries exp(b) moves after the GELU pass so the
            # scalar queue order matches the ACT-table dependency chain).
            st_ = {}             # per-batch live tiles
            pend = []            # deferred-GELU state per batch in group
            grp_tbl_insts = []   # this group's exp/ln ACT instructions
            prev_gelu = None     # last gelu instruction of previous group

            def emit_proj(b):
                xt = []
                for dt_ in range(NDT):
                    t = xtp.tile([128, S], BF16, tag="xt")
                    nc.sync.dma_start(t[:], xT[b, dt_])
                    xt.append(t)
                # vT[h, s] = sum_d v_wT[d, h] * xT[d, s]  (+v_b per-part,
                # applied by the Scalar ACT during the PSUM->SBUF copy)
                vt_sb = []
                for ht in range(NHT):
                    ps = bigps.tile([128, S], F32, tag="bigps")
                    for dt_ in range(NDT):
                        nc.tensor.matmul(
                            ps[:], vw_sb[dt_][:, ht * 128:(ht + 1) * 128],
                            xt[dt_][:],
                            start=(dt_ == 0), stop=(dt_ == NDT - 1))
                    t = vtp.tile([128, S], BF16, tag="vt")
                    nc.scalar.activation(t[:], ps[:], AF.Identity,
                                         bias=vb_sb[ht][:])
                    vt_sb.append(t)
                # kq[s, j] = sum_d x[s, d] * [k_wT | q_wT][d, j]  (+bias)
                kq_sb = []
                for stt in range(NST):
                    ps = bigps.tile([128, 2 * H], F32, tag="bigps")
                    for dt_ in range(NDT):
                        nc.tensor.matmul(
                            ps[:], xt[dt_][:, stt * 128:(stt + 1) * 128],
                            kqw_sb[dt_][:],
                            start=(dt_ == 0), stop=(dt_ == NDT - 1))
                    t = kqp.tile([128, 2 * H], BF16, tag="kq")
                    nc.vector.tensor_add(t[:], ps[:], kqb_sb[:])
                    kq_sb.append(t)
                st_[b] = {"vt": vt_sb, "kq": kq_sb}

            def emit_scores(b):
                s = st_[b]
                kq_sb = s["kq"]
                s["sc"] = []
                for ht in range(NHT):
                    sc = scps.tile([128, H], F32, tag="scps")
                    for stt in range(NST):
                        nc.tensor.matmul(
                            sc[:],
                            kq_sb[stt][:, ht * 128:(ht + 1) * 128],
                            kq_sb[stt][:, H:2 * H],
                            start=(stt == 0), stop=(stt == NST - 1))
                    s["sc"].append(sc)

            def emit_exp(b):
                s = st_[b]
                e_sb = []
                for ht in range(NHT):
                    t = ep.tile([128, H], BF16, tag="e")
                    ei = nc.scalar.activation(
                        t[:], s["sc"][ht][:], AF.Exp,
                        scale=SCALE)
                    grp_tbl_insts.append(ei)
                    e_sb.append(t)
                s["e"] = e_sb

            def emit_denom(b):
                # softmax denom over h (partition dim) via ones-matmuls;
                # approx reciprocal written straight into an f32r tile so
                # the broadcast matmul needs no dtype-convert copy.
                s = st_[b]
                smbc = pairps.tile([128, 2 * H], F32, tag="pair")
                for ht in range(NHT):
                    nc.tensor.matmul(smbc[0:1, 0:H], ones_col, s["e"][ht][:],
                                     start=(ht == 0), stop=(ht == NHT - 1))
                rec_sb = recp.tile([1, H], F32R, tag="rec")
                c = RECIP_APPROX_FAST_CONSTS
                nc.vector._custom_dve(
                    RECIPROCAL_APPROX_FAST, out=rec_sb[:],
                    in0=smbc[0:1, 0:H], s0=c["s0"], s1=c["s1"],
                    imm2=c["imm2"])
                s["smbc"] = smbc
                s["rec"] = rec_sb

            def emit_bc(b):
                s = st_[b]
                bcp = s["smbc"][:, H:2 * H]
                nc.tensor.matmul(bcp, ones_row_r, s["rec"][:],
                                 start=True, stop=True)
                w_sb = []
                for ht in range(NHT):
                    t = wp.tile([128, H], BF16, tag="w")
                    nc.vector.tensor_mul(t[:], s["e"][ht][:], bcp)
                    w_sb.append(t)
                s["w"] = w_sb

            def emit_apply(b):
                # out5[s, g] = sum_h vT[h, s] w[h, g]; +attn_bias; LN stats.
                # p5 tiles pack two st halves per PSUM bank; per-batch
                # mean/var collect into one [128, NST, 2] tile so the
                # ln/rstd pass is 2 ACTs (not 8).
                s = st_[b]
                tl = []
                mva = statp.tile([128, NST, 2], F32, tag="mva")
                for pp in range(NST // 2):
                    p5 = pairps.tile([128, 2 * H], F32, tag="pair")
                    for half in range(2):
                        stt = 2 * pp + half
                        for ht in range(NHT):
                            nc.tensor.matmul(
                                p5[:, half * H:(half + 1) * H],
                                s["vt"][ht][:, stt * 128:(stt + 1) * 128],
                                s["w"][ht][:],
                                start=(ht == 0), stop=(ht == NHT - 1))
                    for half in range(2):
                        stt = 2 * pp + half
                        t_sb = tp.tile([128, H], BF16, tag="t")
                        nc.vector.tensor_add(
                            t_sb[:], p5[:, half * H:(half + 1) * H],
                            ab_sb[stt][:])
                        st6 = statp.tile([128, 6], F32, tag="st6")
                        nc.vector.bn_stats(st6[:], t_sb[:])
                        nc.vector.bn_aggr(mva[:, stt], st6[:])
                        tl.append(t_sb)
                s["t"] = tl
                s["mva"] = mva

            def emit_lnrstd(b):
                # rstd = (var+eps)^-0.5 = exp(-0.5*ln(var+eps)) on all NST
                # tiles at once (strided var columns); nb = -mu*rstd on the
                # otherwise-idle GpSimd engine.
                s = st_[b]
                mva = s["mva"]
                lnv = lnstatp.tile([128, NST], F32, tag="lnv")
                li = nc.scalar.activation(lnv[:], mva[:, :, 1:2], AF.Ln,
                                          bias=eps_sb[:])
                grp_tbl_insts.append(li)
                rstd = lnstatp.tile([128, NST], F32, tag="rstd")
                ri = nc.scalar.activation(rstd[:], lnv[:], AF.Exp,
                                          scale=-0.5)
                grp_tbl_insts.append(ri)
                nb_t = lnstatp.tile([128, NST], F32, tag="nb")
                nc.vector.scalar_tensor_tensor(
                    nb_t[:], mva[:, :, 0:1], -1.0, rstd[:],
                    mybir.AluOpType.mult, mybir.AluOpType.mult)
                pend.append((b, s["t"], rstd, nb_t))
                del st_[b]

            def emit_gelu_group():
                nonlocal prev_gelu, grp_tbl_insts, pend
                if prev_gelu is not None:
                    # keep ACT table phases disjoint across groups
                    for inst in grp_tbl_insts:
                        add_dep_helper(inst.ins, prev_gelu.ins,
                                       sync=False,
                                       reason="act-table grouping")
                last_tbl = grp_tbl_insts[-1]
                grp_tbl_insts = []
                for pb, tl, rstd, nb_t in pend:
                    ck = pb // GB
                    lb = pb % GB
                    for stt in range(NST):
                        act_sb = actp.tile([128, H], BF16, tag="act")
                        if ln_trivial:
                            gi = nc.scalar.activation(
                                act_sb[:], tl[stt][:], AF.Gelu,
                                bias=nb_t[:, stt:stt + 1],
                                scale=rstd[:, stt:stt + 1])
                        else:
                            nrm = tp.tile([128, H], F32, tag="nrm")
                            nc.scalar.activation(
                                nrm[:], tl[stt][:], AF.Identity,
                                bias=nb_t[:, stt:stt + 1],
                                scale=rstd[:, stt:stt + 1])
                            nc.vector.tensor_mul(nrm[:], nrm[:], lng_sb[:])
                            nc.vector.tensor_add(nrm[:], nrm[:], lnb_sb[:])
                            gi = nc.scalar.activation(
                                act_sb[:], nrm[:], AF.Gelu)
                        add_dep_helper(gi.ins, last_tbl.ins,
                                       sync=False,
                                       reason="act-table grouping")
                        # single DMA covering both destination shards;
                        # split across Scalar and GpSimd queues
                        dst = a2a_in[ck][2 * stt:2 * stt + 2, lb]
                        if ck == 0:
                            nc.gpsimd.dma_start(dst, act_sb[:])
                        else:
                            # the AllToAll trigger blocks the GpSimd queue
                            # until the collective completes, so chunk-1
                            # writes must not queue behind chunk-0's trigger
                            nc.sync.dma_start(dst, act_sb[:])
                        prev_gelu = gi
                pend = []

            proj_done = set()
            ow_tiles = [None] * NC_T

            def prefetch_ow(c):
                t = owp.tile([128, D], BF16, tag="ow")
                nc.gpsimd.dma_start(t[:], owT[c])
                ow_tiles[c] = t

            def proj(b):
                if b < NB and b not in proj_done:
                    emit_proj(b)
                    proj_done.add(b)

            for b in range(NB + 1):
                boundary = b > 0 and b % G == 0
                post_boundary = b > 1 and b % G == 1
                if post_boundary:
                    # right after a GELU pass the Scalar queue is still
                    # draining the pass + next exp; give the PE two batches
                    # of exp-independent projection work first
                    proj(b)
                    proj(b + 1)
                    emit_scores(b)
                    emit_exp(b)
                    emit_denom(b - 1)
                    emit_bc(b - 1)
                else:
                    if b > 0:
                        emit_denom(b - 1)
                    proj(b)
                    if b > 0:
                        emit_bc(b - 1)
                    if b < NB:
                        emit_scores(b)
                        if not boundary:
                            emit_exp(b)
                if b > 0:
                    emit_apply(b - 1)
                    emit_lnrstd(b - 1)
                    if boundary:
                        emit_gelu_group()
                        if b < NB:
                            emit_exp(b)
                    if b % GB == 0:
                        ck = b // GB - 1
                        nc.gpsimd.collective_compute(
                            "AllToAll", mybir.AluOpType.bypass,
                            replica_groups=[list(range(N_CORES))],
                            ins=[a2a_in[ck].opt()],
                            outs=[a2a_out[ck].opt()])


            # ---- phase 8: y_part[b, d] = sum_sh actT[sh, b] * owT[sh, d] ----
            # XBAR DMA-transpose slabs [128 batch, CW sh] -> [128 sh, TPS,
            # 128 batch] run on the Scalar queue (idle after attention) so
            # their collective-wait never blocks Sync's ow streaming.
            def emit_slabs(ck, reverse=False):
                order = range(NSLAB - 1, -1, -1) if reverse else range(NSLAB)
                for c8 in order:
                    at = atp.tile([128, TPS, 128], BF16, tag="at")
                    nc.scalar.dma_start_transpose(
                        at[:], a2a_out[ck][0:128, c8 * CW:(c8 + 1) * CW])
                    at_slabs[ck][c8] = at

            emit_slabs(0)
            ypsum = []
            n_fresh = NC_T - OW_BUFS
            # chunk 0: stream all of out_w through owp; the last OW_BUFS
            # tiles stay resident for chunk 1.
            yp_t = bigps.tile([128, D], F32, tag="bigps")
            ypsum.append(yp_t)
            for c in range(NC_T):
                ow_t = owp.tile([128, D], BF16, tag="ow")
                # stream out_w on both free queues so chunk 0 is PE-bound,
                # not DMA-bound (Scalar is idle after the chunk-0 slabs)
                if c % 2 == 0:
                    nc.sync.dma_start(ow_t[:], owT[c])
                else:
                    nc.scalar.dma_start(ow_t[:], owT[c])
                ow_tiles[c] = ow_t
                nc.tensor.matmul(
                    yp_t[:], at_slabs[0][c // TPS][:, c % TPS, :], ow_t[:],
                    start=(c == 0), stop=(c == NC_T - 1))
            # chunk 1: the last OW_BUFS tiles are still resident (process
            # them newest-first so their slots are fully read before the
            # re-streamed tiles rotate in); the re-streamed loads are
            # emitted first so they prefetch during the second AllToAll.
            emit_slabs(1)
            yp_t = bigps.tile([128, D], F32, tag="bigps")
            ypsum.append(yp_t)
            order = (list(range(NC_T - 1, n_fresh - 1, -1))
                     + list(range(n_fresh - 1, -1, -1)))
            for i, c in enumerate(order):
                if c >= n_fresh:
                    ow_t = ow_tiles[c]
                else:
                    ow_t = owp.tile([128, D], BF16, tag="ow")
                    nc.sync.dma_start(ow_t[:], owT[c])
                nc.tensor.matmul(
                    yp_t[:], at_slabs[1][c // TPS][:, c % TPS, :], ow_t[:],
                    start=(i == 0), stop=(i == NC_T - 1))
            for ck in range(CH):
                y_sb = ysbp.tile([128, D], F32, tag="ysb")
                nc.vector.tensor_copy(y_sb[:], ypsum[ck][:])
                nc.sync.dma_start(y_out[ck], y_sb[:])

    nc.compile()
    return nc


_CACHE = {}


def _get_program(ln_trivial):
    if ln_trivial not in _CACHE:
        _CACHE[ln_trivial] = _build(ln_trivial)
    return _CACHE[ln_trivial]


def _prep_inputs(x, k_w, k_b, q_w, q_b, v_w, v_b, attn_bias, ln_g, ln_b,
                 out_w, out_b):
    ln_trivial = bool(np.all(ln_g == 1.0) and np.all(ln_b == 0.0))
    kq_wT = np.ascontiguousarray(
        np.concatenate([k_w.T, q_w.T], axis=1)).reshape(
            NDT, 128, 2 * H).astype(BF)
    v_wT = np.ascontiguousarray(v_w.T).reshape(NDT, 128, H).astype(BF)
    kq_b = np.ascontiguousarray(
        np.tile(np.concatenate([k_b, q_b])[None, :], (128, 1)))
    v_b2 = np.ascontiguousarray(v_b.reshape(NHT, 128, 1))
    ab = np.ascontiguousarray(attn_bias.reshape(NST, 128, H))
    owT_full = np.ascontiguousarray(out_w.T)  # [S*H, D]
    shared = dict(kq_wT=kq_wT, v_wT=v_wT, kq_b=kq_b, v_b2=v_b2, ab=ab,
                  ones_bf=np.ones((128, 128), BF),
                  ones_f32=np.ones((128, 128), np.float32))
    if not ln_trivial:
        shared["lng"] = np.ascontiguousarray(np.tile(ln_g[None, :], (128, 1)))
        shared["lnb"] = np.ascontiguousarray(np.tile(ln_b[None, :], (128, 1)))
    in_maps = []
    for i in range(N_CORES):
        xi = np.ascontiguousarray(
            x[i * NB:(i + 1) * NB].transpose(0, 2, 1)).reshape(
                NB, NDT, 128, S).astype(BF)
        owi = np.ascontiguousarray(
            owT_full[i * SLICE:(i + 1) * SLICE]).reshape(
                NC_T, 128, D).astype(BF)
        m = dict(shared)
        m["xT"] = xi
        m["owT"] = owi
        in_maps.append(m)
    return ln_trivial, in_maps


def _postprocess(res, out_b):
    # each core returns its partial projection [CH, 128, D]; sum over cores
    ysum = np.zeros((CH, N_CORES * GB, D), np.float32)
    for r in res.results:
        ysum += np.asarray(r["y"], np.float32)
    # a2a_out row r of chunk k is global batch (r//GB)*NB + k*GB + (r%GB)
    rr = np.arange(N_CORES * GB)
    y = np.empty((B, D), np.float32)
    for k in range(CH):
        y[(rr // GB) * NB + k * GB + (rr % GB)] = ysum[k]
    y += out_b[None, :]
    return y.reshape(B, 1, D).astype(np.float32)


def kernel(**inputs):
    xs = {k: np.asarray(v, dtype=np.float32) for k, v in inputs.items()}
    ln_trivial, in_maps = _prep_inputs(
        xs["x"], xs["k_w"], xs["k_b"], xs["q_w"], xs["q_b"], xs["v_w"],
        xs["v_b"], xs["attn_bias"], xs["ln_g"], xs["ln_b"], xs["out_w"],
        xs["out_b"])
    nc = _get_program(ln_trivial)
    res = run_bass_kernel_spmd(nc, in_maps, core_ids=list(range(N_CORES)))
    return _postprocess(res, xs["out_b"])
